# revision 1
# baseline (speedup 1.0000x reference)
"""Max pairwise L2 distance between two embedding sets, on 8 Trainium2 cores.

Problem: l [8192, 64] f32, r [8192, 64] f32 -> scalar f32
    out = sqrt(max_ij ||l_i - r_j||^2)

Strategy
--------
The distance matrix has 67M entries; any exact max must examine every one.
On TRN2 the only engines that can read PSUM (where matmul output lands) are
VectorE (1 fp32/lane/cycle @ 0.96 GHz) and ScalarE (1/lane/cycle @ 1.2 GHz),
so the examination is the bottleneck, not the matmul.  We therefore:

1. On host, pick a strong candidate pair (extreme norms / extreme projections)
   and compute its exact distance L.  Pick thr = L - delta where delta bounds
   the bf16 matmul error.  Any entry <= thr cannot beat L.
2. Augment the K dimension so the PE itself computes sq_dist - thr:
      l_aug = [-2*l | lsq_hi lsq_lo 1 1 1]       (K = 69 rows, bf16)
      r_aug = [  r  | 1 1 rsq_hi rsq_lo -thr]
   (norms carried as bf16 hi+lo pairs for accuracy; thr exactly bf16).
3. Shard rows of l across the 8 cores (1024 each); every core streams all of
   r.  Each core runs 128 matmuls of [69,128]x[69,512] into 4-bank PSUM
   groups; VectorE max-reduces odd groups, ScalarE relu+sum-accumulates even
   groups.  A partition-row's partial > 0 iff some entry exceeded thr.
4. Host exactly (float64) recomputes the few flagged rows and returns
   sqrt(max(L, flagged maxima)) - an exact fp32 answer.
"""

import numpy as np
import ml_dtypes

N_CORES = 8
N_L, N_R, DIM = 8192, 8192, 64
K_AUG = 69                      # 64 dims + lsq_hi/lo + rsq_hi/lo + thr
L_COLS = N_L // N_CORES         # 1024 l-rows per core
M_TILE = 128                    # stationary free dim (l rows per matmul)
N_FREE = 512                    # moving free dim (one PSUM bank)
CHUNK = 1024                    # consumer group = 2 banks
BF16 = ml_dtypes.bfloat16

_COMPILED = {}


def _assignment(groups):
    """Bresenham ACT/DVE split (cost-model claim costs are ~equal: ACT
    ~1183ns incl. accumulator read vs DVE ~1192ns). Returns list of bools
    (True = ACT)."""
    n_act = round(groups * 32 / 64)
    out = []
    acc = 0
    for _ in range(groups):
        acc += n_act
        if acc >= groups:
            acc -= groups
            out.append(True)
        else:
            out.append(False)
    assert sum(out) == n_act
    return out


def _ldw_sig(inst):
    """Signature of an InstLdweights' weights operand."""
    w = inst.ins[-1]
    return str(w)


def _dedup_ldweights(ordered_by_block):
    """Drop InstLdweights whose weights match the previous LDW on PE.
    The matmuls still carry the weights AP (dep tracking intact); the PE
    array keeps the loaded stationary across matmuls, so a reload with
    identical weights is pure overhead (~107ns engine time each)."""
    import concourse.mybir as mybir
    removed = 0
    for bb, insts in ordered_by_block.items():
        last_sig = None
        keep = []
        for inst in insts:
            if type(inst).__name__ == "InstLdweights":
                sig = _ldw_sig(inst)
                if sig == last_sig:
                    removed += 1
                    continue
                last_sig = sig
            keep.append(inst)
        if removed:
            insts[:] = keep
    return removed


def _build_nc(l_cols=L_COLS, r_cols=N_R, repeats=1, dyn_loop=False,
              consumer="mixed", chunk=CHUNK, psum_bufs=4, dedup_ldw=False):
    """Build + compile the per-core SPMD program.

    Inputs : l_blk [K_AUG, l_cols] bf16, r_all [K_AUG, r_cols] bf16
             (+ rep_cnt [1,1] i32 when dyn_loop)
    Outputs: dve_part [128, n_dve] f32  (max of sq-thr over group)
             act_part [128, n_act] f32  (sum of relu(sq-thr))

    dyn_loop=True wraps the group loop in a runtime-count For_i (for
    timing: one NEFF, variable work).
    """
    import concourse.tile as tile
    from concourse import bacc, mybir
    from concourse.bass import make_scalar_value, RegisterHandles

    m_tiles = l_cols // M_TILE
    n_chunks = r_cols // chunk
    groups = m_tiles * n_chunks
    if consumer == "mixed":
        assign_act = _assignment(groups)
    elif consumer == "act":
        assign_act = [True] * groups
    elif consumer in ("dve", "none"):
        assign_act = [False] * groups
    else:
        raise ValueError(consumer)
    n_act = sum(assign_act)
    n_dve = groups - n_act

    nc = bacc.Bacc("TRN2", target_bir_lowering=False, debug=False,
                   num_devices=N_CORES)
    bf16 = mybir.dt.bfloat16
    f32 = mybir.dt.float32

    l_in = nc.dram_tensor("l_blk", [K_AUG, l_cols], bf16,
                          kind="ExternalInput").ap()
    r_in = nc.dram_tensor("r_all", [K_AUG, r_cols], bf16,
                          kind="ExternalInput").ap()
    cnt_in = None
    if dyn_loop:
        cnt_in = nc.dram_tensor("rep_cnt", [1, 1], mybir.dt.int32,
                                kind="ExternalInput").ap()
    dve_out = act_out = None
    if n_dve:
        dve_out = nc.dram_tensor("dve_part", [128, n_dve], f32,
                                 kind="ExternalOutput").ap()
    if n_act:
        act_out = nc.dram_tensor("act_part", [128, n_act], f32,
                                 kind="ExternalOutput").ap()

    with tile.TileContext(nc) as tc:
        with (tc.tile_pool(name="io", bufs=1) as io_pool,
              tc.tile_pool(name="psum", bufs=psum_bufs, space="PSUM") as psum_pool,
              tc.tile_pool(name="scratch", bufs=1) as scratch_pool):
            # tiny dummy activation first so the ACT table set loads during
            # the DMA prologue instead of before the first real group
            warm = scratch_pool.tile([128, 1], f32)
            nc.vector.memset(warm[:], 0.0)
            nc.scalar.activation(warm[:], warm[:],
                                 mybir.ActivationFunctionType.Relu)

            l_sb = io_pool.tile([K_AUG, l_cols], bf16)
            # first m-tiles land first so group 0 starts ASAP
            nc.sync.dma_start(l_sb[:, :2 * M_TILE], l_in[:, :2 * M_TILE])
            nc.sync.dma_start(l_sb[:, 2 * M_TILE:], l_in[:, 2 * M_TILE:])
            r_sb = io_pool.tile([K_AUG, r_cols], bf16)
            for ch in range(n_chunks):
                sl = slice(ch * chunk, (ch + 1) * chunk)
                nc.sync.dma_start(r_sb[:, sl], r_in[:, sl])

            dve_sb = act_sb = None
            if n_dve:
                dve_sb = io_pool.tile([128, n_dve], f32, name="dve_sb")
            if n_act:
                act_sb = io_pool.tile([128, n_act], f32, name="act_sb")
            if consumer == "none" and dve_sb is not None:
                nc.vector.memset(dve_sb[:], 0.0)

            def body():
                dve_slot = 0
                act_slot = 0
                # n-chunk outer so the first groups only need r chunk 0
                for g in range(groups):
                    ch, m = divmod(g, m_tiles)
                    ps = psum_pool.tile([128, chunk], f32)
                    for k in range(chunk // N_FREE):
                        ncol = ch * chunk + k * N_FREE
                        nc.tensor.matmul(
                            ps[:, k * N_FREE:(k + 1) * N_FREE],
                            l_sb[:, m * M_TILE:(m + 1) * M_TILE],
                            r_sb[:, ncol:ncol + N_FREE],
                            start=True, stop=True)
                    if consumer == "none":
                        continue
                    if assign_act[g]:
                        # relu in place in PSUM (ScalarE is closest to PSUM;
                        # next matmul start=True clears has_written anyway)
                        nc.scalar.activation(
                            ps[:, :], ps[:, :],
                            mybir.ActivationFunctionType.Relu,
                            accum_out=act_sb[:, act_slot:act_slot + 1])
                        act_slot += 1
                    else:
                        nc.vector.tensor_reduce(
                            dve_sb[:, dve_slot:dve_slot + 1], ps[:, :],
                            axis=mybir.AxisListType.X, op=mybir.AluOpType.max)
                        dve_slot += 1

            if dyn_loop:
                cnt_sb = io_pool.tile([1, 1], mybir.dt.int32)
                nc.sync.dma_start(cnt_sb[:], cnt_in[:])
                regs = []
                for etype in mybir.ALL_ENGINES:
                    eng = nc.engines[etype]
                    reg = eng.alloc_register(f"repcnt_{etype.name}")
                    eng.reg_load(reg, cnt_sb[0:1, 0:1])
                    regs.append(reg)
                end_sv = make_scalar_value(
                    RegisterHandles(regs), min_val=0, max_val=100000)
                with tc.For_i(0, end_sv):
                    body()
            else:
                for _ in range(repeats):
                    body()

            if dve_out is not None:
                nc.sync.dma_start(dve_out[:], dve_sb[:])
            if act_out is not None:
                nc.sync.dma_start(act_out[:], act_sb[:])

    nc.compile()
    return nc


def _get_nc(key=("full", 1)):
    if key not in _COMPILED:
        kind, repeats = key
        _COMPILED[key] = _build_nc(repeats=repeats)
    return _COMPILED[key]


def _candidate_threshold(l64, r64, ln, rn):
    """Exact (float64) max squared distance over a cheap candidate set."""
    cl = set(np.argsort(-ln)[:64].tolist())
    cr = set(np.argsort(-rn)[:64].tolist())
    rng = np.random.default_rng(12345)
    U = rng.standard_normal((16, DIM))
    U /= np.linalg.norm(U, axis=1, keepdims=True)
    pl = l64 @ U.T
    pr = r64 @ U.T
    for k in range(U.shape[0]):
        cl.update(np.argsort(-pl[:, k])[:8].tolist())
        cl.update(np.argsort(pl[:, k])[:8].tolist())
        cr.update(np.argsort(-pr[:, k])[:8].tolist())
        cr.update(np.argsort(pr[:, k])[:8].tolist())
    A = l64[sorted(cl)]
    B = r64[sorted(cr)]
    d2 = ((A * A).sum(1)[:, None] + (B * B).sum(1)[None, :]
          - 2.0 * (A @ B.T))
    return float(d2.max())


def _hi_lo_bf16(x64):
    hi = x64.astype(np.float32).astype(BF16)
    lo = (x64 - hi.astype(np.float64)).astype(np.float32).astype(BF16)
    return hi, lo


def _prepare_inputs(l, r):
    """Returns (l_aug [K_AUG, N_L] bf16, r_aug [K_AUG, N_R] bf16, L, thr)."""
    l64 = l.astype(np.float64)
    r64 = r.astype(np.float64)
    lsq = (l64 * l64).sum(1)
    rsq = (r64 * r64).sum(1)
    ln = np.sqrt(lsq)
    rn = np.sqrt(rsq)

    L = _candidate_threshold(l64, r64, ln, rn)
    # bf16 error bound on device sq-dist: cross term 2^-8 * 2*|l||r|, plus
    # slack for norm hi/lo rounding and fp32 accumulation.
    delta = 2.0 ** -8 * 2.0 * float(ln.max()) * float(rn.max()) + 0.05
    thr = float(np.asarray(L - delta, dtype=np.float32).astype(BF16))

    lsq_hi, lsq_lo = _hi_lo_bf16(lsq)
    rsq_hi, rsq_lo = _hi_lo_bf16(rsq)

    l_aug = np.zeros((K_AUG, N_L), dtype=BF16)
    l_aug[:DIM] = (-2.0 * l.astype(np.float32).T).astype(BF16)
    l_aug[64] = lsq_hi
    l_aug[65] = lsq_lo
    l_aug[66] = BF16(1.0)
    l_aug[67] = BF16(1.0)
    l_aug[68] = BF16(1.0)

    r_aug = np.zeros((K_AUG, N_R), dtype=BF16)
    r_aug[:DIM] = r.astype(np.float32).T.astype(BF16)
    r_aug[64] = BF16(1.0)
    r_aug[65] = BF16(1.0)
    r_aug[66] = rsq_hi
    r_aug[67] = rsq_lo
    r_aug[68] = BF16(-thr)

    return np.ascontiguousarray(l_aug), np.ascontiguousarray(r_aug), L, thr


def _run_device(l_aug, r_aug, nc=None):
    from concourse.bass_utils import run_bass_kernel_spmd
    if nc is None:
        nc = _get_nc()
    in_maps = [
        {"l_blk": np.ascontiguousarray(l_aug[:, c * L_COLS:(c + 1) * L_COLS]),
         "r_all": r_aug}
        for c in range(N_CORES)
    ]
    res = run_bass_kernel_spmd(nc, in_maps, core_ids=list(range(N_CORES)))
    return res.results


def kernel(l_dfa_embeddings, r_dfa_embeddings):
    l = np.asarray(l_dfa_embeddings, dtype=np.float32)
    r = np.asarray(r_dfa_embeddings, dtype=np.float32)
    assert l.shape == (N_L, DIM) and r.shape == (N_R, DIM)

    l_aug, r_aug, L, thr = _prepare_inputs(l, r)
    results = _run_device(l_aug, r_aug)

    l64 = l.astype(np.float64)
    r64 = r.astype(np.float64)
    rsq = (r64 * r64).sum(1)

    m_tiles = L_COLS // M_TILE
    groups = m_tiles * (N_R // CHUNK)
    assign_act = _assignment(groups)
    best = L
    for c in range(N_CORES):
        dve = results[c]["dve_part"]
        act = results[c]["act_part"]
        dve_slot = act_slot = 0
        for g in range(groups):
            if assign_act[g]:
                part = act[:, act_slot]
                act_slot += 1
            else:
                part = dve[:, dve_slot]
                dve_slot += 1
            flagged = np.nonzero(part > 0.0)[0]
            if flagged.size == 0:
                continue
            ch, m = divmod(g, m_tiles)
            cols = slice(ch * CHUNK, (ch + 1) * CHUNK)
            for p in flagged:
                lrow = c * L_COLS + m * M_TILE + int(p)
                d2 = ((l64[lrow] * l64[lrow]).sum() + rsq[cols]
                      - 2.0 * (r64[cols] @ l64[lrow]))
                best = max(best, float(d2.max()))

    return np.float32(np.sqrt(max(best, 0.0)))



# revision 4
# speedup vs baseline: 1.0319x; 1.0319x over previous
"""Max pairwise L2 distance between two embedding sets, on 8 Trainium2 cores.

Problem: l [8192, 64] f32, r [8192, 64] f32 -> scalar f32
    out = sqrt(max_ij ||l_i - r_j||^2)

Strategy
--------
The distance matrix has 67M entries; any exact max must examine every one.
On TRN2 the only engines that can read PSUM (where matmul output lands) are
VectorE (1 fp32/lane/cycle @ 0.96 GHz) and ScalarE (1/lane/cycle @ 1.2 GHz),
so the examination is the bottleneck, not the matmul.  We therefore:

1. On host, pick a strong candidate pair (extreme norms / extreme projections)
   and compute its exact distance L.  Pick thr = L - delta where delta bounds
   the bf16 matmul error.  Any entry <= thr cannot beat L.
2. Augment the K dimension so the PE itself computes sq_dist - thr:
      l_aug = [-2*l | lsq_hi lsq_lo 1 1 1]       (K = 69 rows, bf16)
      r_aug = [  r  | 1 1 rsq_hi rsq_lo -thr]
   (norms carried as bf16 hi+lo pairs for accuracy; thr exactly bf16).
3. Shard rows of l across the 8 cores (1024 each); every core streams all of
   r.  Each core runs 128 matmuls of [69,128]x[69,512] into PSUM, grouped as
   32 four-bank tiles of [128, 2048].  Loop is m-tile-outer so all 16 matmuls
   of an m-tile share one stationary load (redundant InstLdweights are
   dropped post-legalize; their sem waits migrate to the next PE inst).
   ScalarE relu+sum-accumulates 17 of the 32 groups, VectorE max-reduces the
   other 15 (split matches their per-group cost ratio).  A partition-row's
   partial > 0 iff some entry exceeded thr.
4. Host exactly (float64) recomputes the few flagged rows and returns
   sqrt(max(L, flagged maxima)) - an exact fp32 answer.
"""

import numpy as np
import ml_dtypes

N_CORES = 8
N_L, N_R, DIM = 8192, 8192, 64
K_AUG = 69                      # 64 dims + lsq_hi/lo + rsq_hi/lo + thr
L_COLS = N_L // N_CORES         # 1024 l-rows per core
M_TILE = 128                    # stationary free dim (l rows per matmul)
N_FREE = 512                    # moving free dim (one PSUM bank)
CHUNK = 1024                    # consumer group = 2 banks
BF16 = ml_dtypes.bfloat16

_COMPILED = {}

MAX_WAITS_PER_INST = 2          # observed tile/walrus ceiling


def _assignment(groups, n_act=None):
    """Bresenham ACT/DVE split. Per-group cost at chunk 1024: ACT
    (1024+172)/1.2+187 = 1184ns, DVE (1024+120)/0.96 = 1192ns -> ~even
    split. Returns list of bools (True = ACT)."""
    if n_act is None:
        n_act = round(groups * 1192 / (1184 + 1192))
    out = []
    acc = 0
    for _ in range(groups):
        acc += n_act
        if acc >= groups:
            acc -= groups
            out.append(True)
        else:
            out.append(False)
    assert sum(out) == n_act
    return out


def _dedup_ldweights(fn):
    """Drop InstLdweights whose weights AP matches the previous LDW in the
    same basic block (the PE array keeps the stationary loaded across
    matmuls).  Sem waits/updates of dropped LDWs migrate to the next kept
    PE instruction (waits spread under the per-inst capacity)."""
    import bass_rust

    removed = 0
    for bb in fn.blocks:
        insts = bb.instructions
        keep = []
        last_sig = None
        pend_waits = []
        pend_updates = []
        pe_engine = None
        for inst in insts:
            tn = type(inst).__name__
            if tn == "InstLdweights":
                pe_engine = inst.engine
                sig = str(inst.ins[-1])
                if sig == last_sig:
                    si = inst.sync_info
                    if si is not None:
                        pend_waits.extend(list(si.on_wait))
                        pend_updates.extend(list(si.on_update))
                    removed += 1
                    continue
                last_sig = sig
            if (pend_waits or pend_updates) and inst.engine == pe_engine:
                si = inst.sync_info
                cur_w = list(si.on_wait) if si else []
                cur_u = list(si.on_update) if si else []
                room = MAX_WAITS_PER_INST - len(cur_w)
                take, pend_waits = pend_waits[:room], pend_waits[room:]
                inst.sync_info = bass_rust.SyncInfo(
                    on_wait=cur_w + take, on_update=cur_u + pend_updates)
                pend_updates = []
            keep.append(inst)
        if len(keep) != len(insts):
            assert not pend_waits and not pend_updates, (
                "dangling sync from dropped trailing Ldweights")
            insts[:] = keep
    return removed


def _build_nc(l_cols=L_COLS, r_cols=N_R, repeats=1, dyn_loop=False,
              chunk=CHUNK, psum_bufs=4, dedup_ldw=True):
    """Build + compile the per-core SPMD program.

    Inputs : l_blk [K_AUG, l_cols] bf16, r_all [K_AUG, r_cols] bf16
             (+ rep_cnt [1,1] i32 when dyn_loop)
    Outputs: dve_part [128, n_dve] f32  (max of sq-thr over group)
             act_part [128, n_act] f32  (sum of relu(sq-thr))

    dyn_loop=True wraps the group loop in a runtime-count For_i (for
    timing: one NEFF, variable work).
    """
    import concourse.tile as tile
    from concourse import bacc, mybir
    from concourse.bass import make_scalar_value, RegisterHandles

    m_tiles = l_cols // M_TILE
    n_chunks = r_cols // chunk
    groups = m_tiles * n_chunks
    assign_act = _assignment(groups)
    n_act = sum(assign_act)
    n_dve = groups - n_act

    nc = bacc.Bacc("TRN2", target_bir_lowering=False, debug=False,
                   num_devices=N_CORES)
    bf16 = mybir.dt.bfloat16
    f32 = mybir.dt.float32

    l_in = nc.dram_tensor("l_blk", [K_AUG, l_cols], bf16,
                          kind="ExternalInput").ap()
    r_in = nc.dram_tensor("r_all", [K_AUG, r_cols], bf16,
                          kind="ExternalInput").ap()
    cnt_in = None
    if dyn_loop:
        cnt_in = nc.dram_tensor("rep_cnt", [1, 1], mybir.dt.int32,
                                kind="ExternalInput").ap()
    dve_out = act_out = None
    if n_dve:
        dve_out = nc.dram_tensor("dve_part", [128, n_dve], f32,
                                 kind="ExternalOutput").ap()
    if n_act:
        act_out = nc.dram_tensor("act_part", [128, n_act], f32,
                                 kind="ExternalOutput").ap()

    with tile.TileContext(nc) as tc:
        with (tc.tile_pool(name="io", bufs=1) as io_pool,
              tc.tile_pool(name="psum", bufs=psum_bufs, space="PSUM") as psum_pool,
              tc.tile_pool(name="scratch", bufs=1) as scratch_pool):
            # tiny dummy activation first so the ACT table set loads during
            # the DMA prologue instead of before the first real group
            warm = scratch_pool.tile([128, 1], f32)
            nc.vector.memset(warm[:], 0.0)
            nc.scalar.activation(warm[:], warm[:],
                                 mybir.ActivationFunctionType.Relu)

            l_sb = io_pool.tile([K_AUG, l_cols], bf16)
            r_sb = io_pool.tile([K_AUG, r_cols], bf16)
            # group-0 criticals first: l m-tile 0, then r chunk 0
            nc.sync.dma_start(l_sb[:, :M_TILE], l_in[:, :M_TILE])
            nc.sync.dma_start(r_sb[:, :chunk], r_in[:, :chunk])
            nc.sync.dma_start(l_sb[:, M_TILE:], l_in[:, M_TILE:])
            for ch in range(1, n_chunks):
                sl = slice(ch * chunk, (ch + 1) * chunk)
                nc.sync.dma_start(r_sb[:, sl], r_in[:, sl])

            dve_sb = act_sb = None
            if n_dve:
                dve_sb = io_pool.tile([128, n_dve], f32, name="dve_sb")
            if n_act:
                act_sb = io_pool.tile([128, n_act], f32, name="act_sb")

            def body():
                dve_slot = 0
                act_slot = 0
                # m-tile outer: all 16 matmuls of an m-tile share one
                # stationary load (dedup drops the 15 redundant reloads)
                for g in range(groups):
                    m, ch = divmod(g, n_chunks)
                    ps = psum_pool.tile([128, chunk], f32)
                    for k in range(chunk // N_FREE):
                        ncol = ch * chunk + k * N_FREE
                        nc.tensor.matmul(
                            ps[:, k * N_FREE:(k + 1) * N_FREE],
                            l_sb[:, m * M_TILE:(m + 1) * M_TILE],
                            r_sb[:, ncol:ncol + N_FREE],
                            start=True, stop=True)
                    if assign_act[g]:
                        # relu in place in PSUM (ScalarE is closest to PSUM;
                        # next matmul start=True clears has_written anyway)
                        nc.scalar.activation(
                            ps[:, :], ps[:, :],
                            mybir.ActivationFunctionType.Relu,
                            accum_out=act_sb[:, act_slot:act_slot + 1])
                        act_slot += 1
                    else:
                        nc.vector.tensor_reduce(
                            dve_sb[:, dve_slot:dve_slot + 1], ps[:, :],
                            axis=mybir.AxisListType.X, op=mybir.AluOpType.max)
                        dve_slot += 1

            if dyn_loop:
                cnt_sb = io_pool.tile([1, 1], mybir.dt.int32)
                nc.sync.dma_start(cnt_sb[:], cnt_in[:])
                regs = []
                for etype in mybir.ALL_ENGINES:
                    eng = nc.engines[etype]
                    reg = eng.alloc_register(f"repcnt_{etype.name}")
                    eng.reg_load(reg, cnt_sb[0:1, 0:1])
                    regs.append(reg)
                end_sv = make_scalar_value(
                    RegisterHandles(regs), min_val=0, max_val=100000)
                with tc.For_i(0, end_sv):
                    body()
            else:
                for _ in range(repeats):
                    body()

            if dve_out is not None:
                nc.sync.dma_start(dve_out[:], dve_sb[:])
            if act_out is not None:
                nc.sync.dma_start(act_out[:], act_sb[:])

    if dedup_ldw:
        _dedup_ldweights(nc.m.functions[0])
    nc.compile()
    return nc


def _get_nc(key=("full", 1)):
    if key not in _COMPILED:
        kind, repeats = key
        _COMPILED[key] = _build_nc(repeats=repeats)
    return _COMPILED[key]


def _candidate_threshold(l64, r64, ln, rn):
    """Exact (float64) max squared distance over a cheap candidate set."""
    cl = set(np.argsort(-ln)[:64].tolist())
    cr = set(np.argsort(-rn)[:64].tolist())
    rng = np.random.default_rng(12345)
    U = rng.standard_normal((16, DIM))
    U /= np.linalg.norm(U, axis=1, keepdims=True)
    pl = l64 @ U.T
    pr = r64 @ U.T
    for k in range(U.shape[0]):
        cl.update(np.argsort(-pl[:, k])[:8].tolist())
        cl.update(np.argsort(pl[:, k])[:8].tolist())
        cr.update(np.argsort(-pr[:, k])[:8].tolist())
        cr.update(np.argsort(pr[:, k])[:8].tolist())
    A = l64[sorted(cl)]
    B = r64[sorted(cr)]
    d2 = ((A * A).sum(1)[:, None] + (B * B).sum(1)[None, :]
          - 2.0 * (A @ B.T))
    return float(d2.max())


def _hi_lo_bf16(x64):
    hi = x64.astype(np.float32).astype(BF16)
    lo = (x64 - hi.astype(np.float64)).astype(np.float32).astype(BF16)
    return hi, lo


def _prepare_inputs(l, r):
    """Returns (l_aug [K_AUG, N_L] bf16, r_aug [K_AUG, N_R] bf16, L, thr)."""
    l64 = l.astype(np.float64)
    r64 = r.astype(np.float64)
    lsq = (l64 * l64).sum(1)
    rsq = (r64 * r64).sum(1)
    ln = np.sqrt(lsq)
    rn = np.sqrt(rsq)

    L = _candidate_threshold(l64, r64, ln, rn)
    # bf16 error bound on device sq-dist: cross term 2^-8 * 2*|l||r|, plus
    # slack for norm hi/lo rounding and fp32 accumulation.
    delta = 2.0 ** -8 * 2.0 * float(ln.max()) * float(rn.max()) + 0.05
    thr = float(np.asarray(L - delta, dtype=np.float32).astype(BF16))

    lsq_hi, lsq_lo = _hi_lo_bf16(lsq)
    rsq_hi, rsq_lo = _hi_lo_bf16(rsq)

    l_aug = np.zeros((K_AUG, N_L), dtype=BF16)
    l_aug[:DIM] = (-2.0 * l.astype(np.float32).T).astype(BF16)
    l_aug[64] = lsq_hi
    l_aug[65] = lsq_lo
    l_aug[66] = BF16(1.0)
    l_aug[67] = BF16(1.0)
    l_aug[68] = BF16(1.0)

    r_aug = np.zeros((K_AUG, N_R), dtype=BF16)
    r_aug[:DIM] = r.astype(np.float32).T.astype(BF16)
    r_aug[64] = BF16(1.0)
    r_aug[65] = BF16(1.0)
    r_aug[66] = rsq_hi
    r_aug[67] = rsq_lo
    r_aug[68] = BF16(-thr)

    return np.ascontiguousarray(l_aug), np.ascontiguousarray(r_aug), L, thr


def _run_device(l_aug, r_aug, nc=None):
    from concourse.bass_utils import run_bass_kernel_spmd
    if nc is None:
        nc = _get_nc()
    in_maps = [
        {"l_blk": np.ascontiguousarray(l_aug[:, c * L_COLS:(c + 1) * L_COLS]),
         "r_all": r_aug}
        for c in range(N_CORES)
    ]
    res = run_bass_kernel_spmd(nc, in_maps, core_ids=list(range(N_CORES)))
    return res.results


def kernel(l_dfa_embeddings, r_dfa_embeddings):
    l = np.asarray(l_dfa_embeddings, dtype=np.float32)
    r = np.asarray(r_dfa_embeddings, dtype=np.float32)
    assert l.shape == (N_L, DIM) and r.shape == (N_R, DIM)

    l_aug, r_aug, L, thr = _prepare_inputs(l, r)
    results = _run_device(l_aug, r_aug)

    l64 = l.astype(np.float64)
    r64 = r.astype(np.float64)
    rsq = (r64 * r64).sum(1)

    n_chunks = N_R // CHUNK
    m_tiles = L_COLS // M_TILE
    groups = m_tiles * n_chunks
    assign_act = _assignment(groups)
    best = L
    for c in range(N_CORES):
        dve = results[c].get("dve_part")
        act = results[c].get("act_part")
        dve_slot = act_slot = 0
        for g in range(groups):
            if assign_act[g]:
                part = act[:, act_slot]
                act_slot += 1
            else:
                part = dve[:, dve_slot]
                dve_slot += 1
            flagged = np.nonzero(part > 0.0)[0]
            if flagged.size == 0:
                continue
            m, ch = divmod(g, n_chunks)
            cols = slice(ch * CHUNK, (ch + 1) * CHUNK)
            for p in flagged:
                lrow = c * L_COLS + m * M_TILE + int(p)
                d2 = ((l64[lrow] * l64[lrow]).sum() + rsq[cols]
                      - 2.0 * (r64[cols] @ l64[lrow]))
                best = max(best, float(d2.max()))

    return np.float32(np.sqrt(max(best, 0.0)))


# revision 9
# speedup vs baseline: 12.4050x; 12.0221x over previous
"""Max pairwise L2 distance between two embedding sets, on 8 Trainium2 cores.

Problem: l [8192, 64] f32, r [8192, 64] f32 -> scalar f32
    out = sqrt(max_ij ||l_i - r_j||^2)

Strategy
--------
1. On host, pick a strong candidate pair (extreme norms / extreme projections)
   and compute its exact distance^2 L.  Pick thr = L - delta where delta
   bounds the bf16 matmul error.  Any entry <= thr cannot beat L.
2. Norm pruning (exact, triangle inequality): d2(i,j) <= (|l_i|+|r_j|)^2.
   Sort l rows and r columns by norm.  For an l-tile of 128 rows with max
   norm lam, every sorted-r column with |r_j| < sqrt(thr) - lam is provably
   below thr; only a SUFFIX of the sorted r columns needs checking.  On this
   data that prunes ~87% of all pairs.
3. The 64 l-tiles are ranked by suffix length and dealt column-major onto an
   8 cores x 8 slots grid, so slot b has a fixed chunk count C[b] on every
   core (SPMD-uniform program; r is shared).  Slot b streams the suffix
   window of C[b]*512 sorted-r columns.
4. Augment the contraction so the PE emits sq_dist - thr directly:
      l_aug = [-2*l | lsq_hi lsq_lo 1 1 1]       (K = 69 rows, bf16)
      r_aug = [  r  | 1 1 rsq_hi rsq_lo -thr]
   ScalarE relu+sum-accumulates or VectorE max-reduces each PSUM group
   (split by measured per-group cost); a partition-row's partial > 0 iff
   some entry exceeded thr.
5. Host exactly (float64) recomputes the few flagged rows and returns
   sqrt(max(L, flagged maxima)) - an exact fp32 answer.
"""

import numpy as np
import ml_dtypes

N_CORES = 8
N_L, N_R, DIM = 8192, 8192, 64
K_AUG = 69                      # 64 dims + lsq_hi/lo + rsq_hi/lo + thr
L_COLS = N_L // N_CORES         # 1024 l-rows per core
M_TILE = 128                    # stationary free dim (l rows per matmul)
N_FREE = 512                    # moving free dim (one PSUM bank)
N_SLOTS = L_COLS // M_TILE      # 8 l-tiles per core
BF16 = ml_dtypes.bfloat16

_COMPILED = {}

MAX_WAITS_PER_INST = 2          # observed tile/walrus ceiling


# measured per-group consumer costs on this part (ns, FD columns per group)
def _act_cost(fd):
    return (fd + 31) / 1.2


def _dve_cost(fd):
    return (fd + 120) / 0.96


def _schedule(C):
    """Group/engine schedule for per-slot chunk counts C (len 8).

    Slots run in ascending-C order (cheap slots first, so the deep r-window
    DMA can still be in flight).  Within a slot, chunks pair into 1024-wide
    groups plus one ragged 512 if odd.  Each group goes to whichever of
    ACT/DVE has less accumulated predicted time.
    Returns (slot_order, groups, n_act, n_dve); groups = [(slot, col_off,
    width, is_act, out_slot)] in program order; col_off is relative to the
    slot's window start.
    """
    slot_order = sorted(range(len(C)), key=lambda b: (C[b], b))
    raw = []
    for b in slot_order:
        off = 0
        n = C[b]
        while n >= 2:
            raw.append((b, off, 2 * N_FREE))
            off += 2 * N_FREE
            n -= 2
        if n:
            raw.append((b, off, N_FREE))
    t_act = t_dve = 0.0
    n_act = n_dve = 0
    groups = []
    for b, off, w in raw:
        if t_act + _act_cost(w) <= t_dve + _dve_cost(w):
            groups.append((b, off, w, True, n_act))
            t_act += _act_cost(w)
            n_act += 1
        else:
            groups.append((b, off, w, False, n_dve))
            t_dve += _dve_cost(w)
            n_dve += 1
    return slot_order, groups, n_act, n_dve


def _dedup_ldweights(fn):
    """Drop InstLdweights whose weights AP matches the previous LDW in the
    same basic block (the PE array keeps the stationary loaded across
    matmuls).  Sem waits/updates of dropped LDWs migrate to the next kept
    PE instruction."""
    import bass_rust

    removed = 0
    for bb in fn.blocks:
        insts = bb.instructions
        keep = []
        last_sig = None
        pend_waits = []
        pend_updates = []
        pe_engine = None
        for inst in insts:
            tn = type(inst).__name__
            if tn == "InstLdweights":
                pe_engine = inst.engine
                sig = str(inst.ins[-1])
                if sig == last_sig:
                    si = inst.sync_info
                    if si is not None:
                        pend_waits.extend(list(si.on_wait))
                        pend_updates.extend(list(si.on_update))
                    removed += 1
                    continue
                last_sig = sig
            if (pend_waits or pend_updates) and inst.engine == pe_engine:
                si = inst.sync_info
                cur_w = list(si.on_wait) if si else []
                cur_u = list(si.on_update) if si else []
                room = MAX_WAITS_PER_INST - len(cur_w)
                take, pend_waits = pend_waits[:room], pend_waits[room:]
                inst.sync_info = bass_rust.SyncInfo(
                    on_wait=cur_w + take, on_update=cur_u + pend_updates)
                pend_updates = []
            keep.append(inst)
        if len(keep) != len(insts):
            assert not pend_waits and not pend_updates, (
                "dangling sync from dropped trailing Ldweights")
            insts[:] = keep
    return removed


def _build_nc(C, repeats=1, dyn_loop=False, dedup_ldw=True):
    """Build + compile the per-core SPMD program for chunk schedule C.

    Inputs : l_blk [K_AUG, L_COLS] bf16 (slot-major sorted l tiles),
             r_all [K_AUG, N_R] bf16 (norm-sorted r)
             (+ rep_cnt [1,1] i32 when dyn_loop)
    Outputs: dve_part [128, n_dve] f32, act_part [128, n_act] f32
    """
    import concourse.tile as tile
    from concourse import bacc, mybir
    from concourse.bass import make_scalar_value, RegisterHandles

    C = tuple(C)
    slot_order, groups, n_act, n_dve = _schedule(C)

    nc = bacc.Bacc("TRN2", target_bir_lowering=False, debug=False,
                   num_devices=N_CORES)
    bf16 = mybir.dt.bfloat16
    f32 = mybir.dt.float32

    l_in = nc.dram_tensor("l_blk", [K_AUG, L_COLS], bf16,
                          kind="ExternalInput").ap()
    r_in = nc.dram_tensor("r_all", [K_AUG, N_R], bf16,
                          kind="ExternalInput").ap()
    cnt_in = None
    if dyn_loop:
        cnt_in = nc.dram_tensor("rep_cnt", [1, 1], mybir.dt.int32,
                                kind="ExternalInput").ap()
    dve_out = act_out = None
    if n_dve:
        dve_out = nc.dram_tensor("dve_part", [128, n_dve], f32,
                                 kind="ExternalOutput").ap()
    if n_act:
        act_out = nc.dram_tensor("act_part", [128, n_act], f32,
                                 kind="ExternalOutput").ap()

    with tile.TileContext(nc) as tc:
        with (tc.tile_pool(name="io", bufs=1) as io_pool,
              tc.tile_pool(name="psum", bufs=4, space="PSUM") as psum_pool,
              tc.tile_pool(name="scratch", bufs=1) as scratch_pool):
            # dummy activation so the ACT table set loads during the DMA
            # prologue instead of before the first real group
            warm = scratch_pool.tile([128, 1], f32)
            nc.vector.memset(warm[:], 0.0)
            nc.scalar.activation(warm[:], warm[:],
                                 mybir.ActivationFunctionType.Relu)

            l_sb = io_pool.tile([K_AUG, L_COLS], bf16)
            r_sb = io_pool.tile([K_AUG, N_R], bf16)
            # criticals first: first-processed slot's l tile, then r suffix
            # blocks top-down (all windows are suffixes of sorted r)
            first = slot_order[0]
            sl0 = slice(first * M_TILE, (first + 1) * M_TILE)
            nc.sync.dma_start(l_sb[:, sl0], l_in[:, sl0])
            nblk = N_R // 1024
            for k in range(nblk - 1, -1, -1):
                sl = slice(k * 1024, (k + 1) * 1024)
                nc.sync.dma_start(r_sb[:, sl], r_in[:, sl])
                if k == nblk - 1:
                    rest = [b for b in range(N_SLOTS) if b != first]
                    lo = min(rest) * M_TILE
                    hi = (max(rest) + 1) * M_TILE
                    nc.sync.dma_start(l_sb[:, lo:hi], l_in[:, lo:hi])

            dve_sb = act_sb = None
            if n_dve:
                dve_sb = io_pool.tile([128, n_dve], f32, name="dve_sb")
            if n_act:
                act_sb = io_pool.tile([128, n_act], f32, name="act_sb")

            def body():
                for b, off, w, is_act, out_slot in groups:
                    win = N_R - C[b] * N_FREE
                    ps = psum_pool.tile([128, 2 * N_FREE], f32)
                    for k in range(w // N_FREE):
                        ncol = win + off + k * N_FREE
                        nc.tensor.matmul(
                            ps[:, k * N_FREE:(k + 1) * N_FREE],
                            l_sb[:, b * M_TILE:(b + 1) * M_TILE],
                            r_sb[:, ncol:ncol + N_FREE],
                            start=True, stop=True)
                    if is_act:
                        nc.scalar.activation(
                            ps[:, :w], ps[:, :w],
                            mybir.ActivationFunctionType.Relu,
                            accum_out=act_sb[:, out_slot:out_slot + 1])
                    else:
                        nc.vector.tensor_reduce(
                            dve_sb[:, out_slot:out_slot + 1], ps[:, :w],
                            axis=mybir.AxisListType.X, op=mybir.AluOpType.max)

            if dyn_loop:
                cnt_sb = io_pool.tile([1, 1], mybir.dt.int32)
                nc.sync.dma_start(cnt_sb[:], cnt_in[:])
                regs = []
                for etype in mybir.ALL_ENGINES:
                    eng = nc.engines[etype]
                    reg = eng.alloc_register(f"repcnt_{etype.name}")
                    eng.reg_load(reg, cnt_sb[0:1, 0:1])
                    regs.append(reg)
                end_sv = make_scalar_value(
                    RegisterHandles(regs), min_val=0, max_val=100000)
                with tc.For_i(0, end_sv):
                    body()
            else:
                for _ in range(repeats):
                    body()

            if dve_out is not None:
                nc.sync.dma_start(dve_out[:], dve_sb[:])
            if act_out is not None:
                nc.sync.dma_start(act_out[:], act_sb[:])

    if dedup_ldw:
        _dedup_ldweights(nc.m.functions[0])
    nc.compile()
    return nc


def _get_nc(C, dyn_loop=False):
    key = (tuple(C), dyn_loop)
    if key not in _COMPILED:
        _COMPILED[key] = _build_nc(C, dyn_loop=dyn_loop)
    return _COMPILED[key]


def _candidate_threshold(l64, r64, ln, rn):
    """Exact (float64) max squared distance over a cheap candidate set."""
    cl = set(np.argsort(-ln)[:64].tolist())
    cr = set(np.argsort(-rn)[:64].tolist())
    rng = np.random.default_rng(12345)
    U = rng.standard_normal((16, DIM))
    U /= np.linalg.norm(U, axis=1, keepdims=True)
    pl = l64 @ U.T
    pr = r64 @ U.T
    for k in range(U.shape[0]):
        cl.update(np.argsort(-pl[:, k])[:8].tolist())
        cl.update(np.argsort(pl[:, k])[:8].tolist())
        cr.update(np.argsort(-pr[:, k])[:8].tolist())
        cr.update(np.argsort(pr[:, k])[:8].tolist())
    A = l64[sorted(cl)]
    B = r64[sorted(cr)]
    d2 = ((A * A).sum(1)[:, None] + (B * B).sum(1)[None, :]
          - 2.0 * (A @ B.T))
    return float(d2.max())


def _hi_lo_bf16(x64):
    hi = x64.astype(np.float32).astype(BF16)
    lo = (x64 - hi.astype(np.float64)).astype(np.float32).astype(BF16)
    return hi, lo


def _prepare_inputs(l, r):
    """Sort, prune, augment.  Returns a dict with per-core l blocks, the
    shared sorted r_aug, the chunk schedule C, and decode tables."""
    l64 = l.astype(np.float64)
    r64 = r.astype(np.float64)
    lsq = (l64 * l64).sum(1)
    rsq = (r64 * r64).sum(1)
    ln = np.sqrt(lsq)
    rn = np.sqrt(rsq)

    L = _candidate_threshold(l64, r64, ln, rn)
    # bf16 error bound on device sq-dist: cross term 2^-8 * 2*|l||r|, plus
    # slack for norm hi/lo rounding and fp32 accumulation.
    delta = 2.0 ** -8 * 2.0 * float(ln.max()) * float(rn.max()) + 0.05
    thr = float(np.asarray(L - delta, dtype=np.float32).astype(BF16))

    PL = np.argsort(ln, kind="stable")
    PR = np.argsort(rn, kind="stable")
    rs = rn[PR]

    n_tiles = N_L // M_TILE
    lam = ln[PL].reshape(n_tiles, M_TILE).max(1)
    # exact first needed sorted-r column per tile: (lam + |r|)^2 >= thr
    S = np.searchsorted(rs, np.sqrt(thr) - lam)
    chunks = np.ceil((N_R - S) / N_FREE).astype(int)

    # deal tiles column-major by descending need: slot b of core c gets
    # rank b*8+c; C[b] = max need in band b (suffix windows cover members)
    order = np.argsort(-chunks, kind="stable")
    C = tuple(int(chunks[order[b * N_CORES:(b + 1) * N_CORES]].max())
              for b in range(N_SLOTS))
    tile_of = [[int(order[b * N_CORES + c]) for b in range(N_SLOTS)]
               for c in range(N_CORES)]

    lsq_hi, lsq_lo = _hi_lo_bf16(lsq[PL])
    rsq_hi, rsq_lo = _hi_lo_bf16(rsq[PR])

    l_aug = np.zeros((K_AUG, N_L), dtype=BF16)
    l_aug[:DIM] = (-2.0 * l.astype(np.float32)[PL].T).astype(BF16)
    l_aug[64] = lsq_hi
    l_aug[65] = lsq_lo
    l_aug[66] = BF16(1.0)
    l_aug[67] = BF16(1.0)
    l_aug[68] = BF16(1.0)

    r_aug = np.zeros((K_AUG, N_R), dtype=BF16)
    r_aug[:DIM] = r.astype(np.float32)[PR].T.astype(BF16)
    r_aug[64] = BF16(1.0)
    r_aug[65] = BF16(1.0)
    r_aug[66] = rsq_hi
    r_aug[67] = rsq_lo
    r_aug[68] = BF16(-thr)

    l_blks = []
    for c in range(N_CORES):
        cols = np.concatenate(
            [np.arange(t * M_TILE, (t + 1) * M_TILE) for t in tile_of[c]])
        l_blks.append(np.ascontiguousarray(l_aug[:, cols]))

    return {
        "l_blks": l_blks,
        "r_aug": np.ascontiguousarray(r_aug),
        "C": C,
        "tile_of": tile_of,
        "PL": PL,
        "PR": PR,
        "L": L,
        "thr": thr,
    }


def _run_device(prep, nc=None):
    from concourse.bass_utils import run_bass_kernel_spmd
    if nc is None:
        nc = _get_nc(prep["C"])
    in_maps = [
        {"l_blk": prep["l_blks"][c], "r_all": prep["r_aug"]}
        for c in range(N_CORES)
    ]
    res = run_bass_kernel_spmd(nc, in_maps, core_ids=list(range(N_CORES)))
    return res.results


def kernel(l_dfa_embeddings, r_dfa_embeddings):
    l = np.asarray(l_dfa_embeddings, dtype=np.float32)
    r = np.asarray(r_dfa_embeddings, dtype=np.float32)
    assert l.shape == (N_L, DIM) and r.shape == (N_R, DIM)

    prep = _prepare_inputs(l, r)
    results = _run_device(prep)

    l64 = l.astype(np.float64)
    r64 = r.astype(np.float64)
    rsq = (r64 * r64).sum(1)

    _, groups, n_act, n_dve = _schedule(prep["C"])
    PL = prep["PL"]
    best = prep["L"]
    for c in range(N_CORES):
        dve = results[c].get("dve_part")
        act = results[c].get("act_part")
        for b, off, w, is_act, out_slot in groups:
            part = (act[:, out_slot] if is_act else dve[:, out_slot])
            flagged = np.nonzero(part > 0.0)[0]
            if flagged.size == 0:
                continue
            t = prep["tile_of"][c][b]
            for p in flagged:
                lrow = int(PL[t * M_TILE + int(p)])
                d2 = ((l64[lrow] * l64[lrow]).sum() + rsq
                      - 2.0 * (r64 @ l64[lrow]))
                best = max(best, float(d2.max()))

    return np.float32(np.sqrt(max(best, 0.0)))
